# revision 43
# baseline (speedup 1.0000x reference)
"""Trainium2 Bass kernel for nn_Aggregator (segment_reduce):
res[b,d] = sum_n mask[b,n] * (nodes@Wt.T + bt)[n,d] * sigmoid(nodes@Wg.T + bg)[n,d]

Sharding: nodes and owner_masks split along N across 8 NeuronCores; params
replicated; per-core partial [B,D] summed on host.

All matmuls are fp8e4m3 DoubleRow (0.5 cyc/row, 256-deep contraction in one
pass) and the node/mask streams ship as fp8 (masks are 0/1 -> exact), so
HBM traffic halves to ~12.8MB/core vs a bf16 layout. Per pair of 128-node
subchunks:

    dgd = ndf.T@WtT + ones.T@[bt;bt_lo]     (PE: 2 node matmuls open the
                                             bank's group, wide fp8 hi/lo
                                             bias rows close it — so the
                                             first fill never waits on the
                                             bias constants landing)
    dgg = ndf.T@WgT                          (PE; bg rides the ACT bias)
    gt  = sigmoid(dgg + bg)                  (ACT, PSUM -> SBUF bf16)
    pr  = dgd * gt -> fp8                    (DVE - the bottleneck engine)
    res[b,:] += maskf8.T @ pr                (PE DoubleRow over the pair)

PSUM: 3 persistent data banks + 4 persistent gates banks, plus res0|res1
sharing one bank as a single accumulation group (only the very first
matmul uses start=True; the per-element has_written bits make the first
write to each untouched region an overwrite). Tiles are distinct
persistent allocations, NOT pool-rotated: PSUM pool release/realloc
churn crashes the hardware run (engine error at output fetch), while
slicing one big tile serializes the tile scheduler; distinct tiles give
precise WAR deps and pipeline cleanly.

Weight and mask DMAs go through the GPSIMD SWDGE queue so the node
stream owns the HWDGE descriptor pipeline; mask matmuls are emitted 2
pairs late so the in-order PE queue never parks on a res matmul whose
pr the DVE hasn't produced yet; warmup matmuls on a zeroed tile keep
the PE clock at 2.4 GHz from the first real matmul.
"""

import os
import sys
from contextlib import ExitStack

import numpy as np

sys.path.insert(0, "/opt/trn_rl_repo")

import concourse.bass as bass  # noqa: E402
import concourse.tile as tile  # noqa: E402
from concourse import bacc, mybir  # noqa: E402
from concourse.bass_utils import run_bass_kernel_spmd  # noqa: E402

N, D_IN, D_OUT, B = 200000, 256, 256, 256
NCORES = 8
CHUNK = 128
GROUP = 3584
NSH = 25088          # padded nodes per core (= 196 * 128 = 7 * 3584)
NGROUPS = NSH // GROUP
SUBS = GROUP // CHUNK
PAIRS = SUBS // 2

F32 = mybir.dt.float32
BF16 = mybir.dt.bfloat16
FP8 = mybir.dt.float8e4
DR = mybir.MatmulPerfMode.DoubleRow

_BUILT = {}
_LAST_BG_SCALAR = 1.0
WARMUPS = int(os.environ.get("BASS_AGG_WARMUPS", "50"))
SWQ = os.environ.get("BASS_AGG_SWQ", "1") == "1"


def _build(bg_scalar):
    nc = bacc.Bacc("TRN2", target_bir_lowering=False, debug=False,
                   num_devices=NCORES)

    ndf = nc.dram_tensor("ndf", [NGROUPS, 128, 2 * GROUP], FP8,
                         kind="ExternalInput").ap()
    mkf = nc.dram_tensor("mkf", [NGROUPS, 128, SUBS * 256], FP8,
                         kind="ExternalInput").ap()
    wcb = nc.dram_tensor("wcb", [128, 1024], FP8, kind="ExternalInput").ap()
    btf = nc.dram_tensor("btf", [1, 2048], FP8, kind="ExternalInput").ap()
    out_res = nc.dram_tensor("res", [B, D_OUT], F32, kind="ExternalOutput").ap()

    SIG = mybir.ActivationFunctionType.Sigmoid
    cq = nc.gpsimd if SWQ else nc.scalar   # const/mask DMA queue
    mq = nc.gpsimd if SWQ else nc.sync

    with tile.TileContext(nc) as tc, ExitStack() as ctx:
        const = ctx.enter_context(tc.tile_pool(name="const", bufs=1))
        gio = ctx.enter_context(tc.tile_pool(name="gio", bufs=2))
        work = ctx.enter_context(tc.tile_pool(name="work", bufs=2))
        pps = ctx.enter_context(tc.tile_pool(name="pps", bufs=1, space="PSUM"))
        rps = ctx.enter_context(tc.tile_pool(name="rps", bufs=1, space="PSUM"))

        # consts first on the SWDGE queue (tiny; land ~2us), then the first
        # node slices own the HWDGE descriptor pipeline from t=0
        wcb_s = const.tile([128, 1024], FP8)
        cq.dma_start(wcb_s[:], wcb[:])
        btf_s = const.tile([1, 2048], FP8)
        nc.scalar.dma_start(btf_s[:], btf[:])
        ones_s = const.tile([1, 256], FP8)
        nc.vector.memset(ones_s[:], 1.0)

        # group 0 node slices ramp 256 -> 1024 nodes so the first pair's
        # operands land as early as possible while later slices amortize
        # HWDGE descriptor generation
        G0_CUTS = [0, 256, 512, 1024, 1536, 2560, 3584]
        g0_nd = gio.tile([128, 2 * GROUP], FP8, tag="nd")
        g0_nd3 = g0_nd[:].rearrange("p (k n) -> p k n", k=2)
        nc.sync.dma_start(g0_nd3[:, :, G0_CUTS[0]:G0_CUTS[1]],
                          ndf[0].rearrange("p (k n) -> p k n", k=2)
                          [:, :, G0_CUTS[0]:G0_CUTS[1]])
        g0_mk = gio.tile([128, SUBS * 256], FP8, tag="mk")
        W = SUBS * 256
        NSP0 = 7
        mq.dma_start(g0_mk[:, 0:W // NSP0], mkf[0][:, 0:W // NSP0])

        wtf3 = wcb_s[:, 0:512].rearrange("p (k d) -> p k d", k=2)
        wgf3 = wcb_s[:, 512:1024].rearrange("p (k d) -> p k d", k=2)
        btf3 = btf_s[:].rearrange("o (r d) -> o r d", r=2)
        ones3 = ones_s[:].rearrange("o (r m) -> o r m", r=2)

        resC = rps.tile([128, 2 * D_OUT], F32)
        res0 = resC[:, 0:D_OUT]
        res1 = resC[:, D_OUT:2 * D_OUT]
        dgd_t = [pps.tile([128, 512], F32, name=f"dgdp{i}") for i in range(3)]
        dgg_t = [pps.tile([128, 512], F32, name=f"dggp{i}") for i in range(4)]

        wz = const.tile([128, 64], BF16)
        nc.vector.memset(wz[:], 0.0)
        for _ in range(WARMUPS):
            nc.tensor.matmul(resC[0:64, 0:64], wz[:], wz[:],
                             start=True, stop=True)

        pending = []
        for g in range(NGROUPS):
            nsp = NSP0 if g == 0 else 2
            nd_s = g0_nd if g == 0 else gio.tile([128, 2 * GROUP], FP8,
                                                 tag="nd")
            mk_s = g0_mk if g == 0 else gio.tile([128, SUBS * 256], FP8,
                                                 tag="mk")
            nd3 = nd_s[:].rearrange("p (k n) -> p k n", k=2)
            ndg = ndf[g].rearrange("p (k n) -> p k n", k=2)
            if g == 0:
                for q in range(1, len(G0_CUTS) - 1):
                    lo, hi = G0_CUTS[q], G0_CUTS[q + 1]
                    nc.sync.dma_start(nd3[:, :, lo:hi], ndg[:, :, lo:hi])
                for q in range(1, NSP0):
                    lo, hi = q * W // NSP0, (q + 1) * W // NSP0
                    mq.dma_start(mk_s[:, lo:hi], mkf[g][:, lo:hi])
            else:
                for q in range(nsp):
                    lo, hi = q * GROUP // nsp, (q + 1) * GROUP // nsp
                    nc.sync.dma_start(nd3[:, :, lo:hi], ndg[:, :, lo:hi])
                    lo, hi = q * W // nsp, (q + 1) * W // nsp
                    mq.dma_start(mk_s[:, lo:hi], mkf[g][:, lo:hi])

            mk4 = mk_s[:].rearrange("p (pr j b) -> p pr j b", pr=PAIRS, j=2)
            prg = work.tile([128, SUBS * 256], FP8, tag="prg")

            for pair in range(PAIRS):
                s0 = 2 * pair
                dgd = dgd_t[pair % 3][:]
                dgg = dgg_t[(g * PAIRS + pair) % 4][:]
                # node matmuls first (the j==0 start=True clears the bank's
                # has_written bits; j==1's start=False is an overwrite on its
                # untouched region), then the wide hi/lo bias rows accumulate
                # over both regions — so the first PE work doesn't wait for
                # the bias constants to land
                for j in range(2):
                    nds = nd3[:, :, (s0 + j) * 128:(s0 + j + 1) * 128]
                    o = j * 256
                    nc.tensor.matmul(dgd[:, o:o + 256], nds, wtf3,
                                     start=(j == 0), stop=False, perf_mode=DR,
                                     skip_group_check=True)
                    nc.tensor.matmul(dgg[:, o:o + 256], nds, wgf3,
                                     start=(j == 0),
                                     stop=(bg_scalar is not None and j == 1),
                                     perf_mode=DR, skip_group_check=True)
                nc.tensor.matmul(dgd, ones3, btf3[:, :, 0:512],
                                 start=False, stop=True, perf_mode=DR,
                                 skip_group_check=True)
                if bg_scalar is None:
                    nc.tensor.matmul(dgg, ones3, btf3[:, :, 512:1024],
                                     start=False, stop=True, perf_mode=DR,
                                     skip_group_check=True)

                # ACT: sigmoid evacuates the gates bank (bg as scalar bias);
                # gt in SBUF with 4 bufs keeps the dgg WAR loop short
                gt = work.tile([128, 512], BF16, tag="gt", bufs=4)
                nc.scalar.activation(gt[:], dgg, SIG,
                                     bias=float(bg_scalar or 0.0), scale=1.0)
                # DVE: the fused multiply -> fp8 product
                nc.vector.tensor_mul(prg[:, s0 * 256:(s0 + 2) * 256],
                                     dgd, gt[:])

                first = (g == 0 and pair == 0)
                last = (g == NGROUPS - 1 and pair == PAIRS - 1)
                prp = prg[:, pair * 512:(pair + 1) * 512].rearrange(
                    "p (j d) -> p j d", j=2)

                def emit(prp=prp, mk4=mk4, pair=pair, first=first,
                         last=last):
                    nc.tensor.matmul(res0, mk4[:, pair, :, 0:128], prp,
                                     start=first, stop=False, perf_mode=DR,
                                     skip_group_check=True)
                    nc.tensor.matmul(res1, mk4[:, pair, :, 128:256], prp,
                                     start=False, stop=last, perf_mode=DR,
                                     skip_group_check=True)
                pending.append(emit)
                while len(pending) > 2:
                    pending.pop(0)()

        while pending:
            pending.pop(0)()

        # split output: each half's DMA issues right after its copy so the
        # first DMA's descriptor generation overlaps the second copy
        rs = work.tile([128, 2 * D_OUT], F32, tag="rout")
        out3d = out_res.rearrange("(h b) d -> b h d", h=2)
        nc.scalar.copy(rs[:, 0:256], res0)
        nc.sync.dma_start(out3d[:, 0, :], rs[:, 0:256])
        nc.vector.tensor_copy(rs[:, 256:512], res1)
        nc.sync.dma_start(out3d[:, 1, :], rs[:, 256:512])

    nc.compile()
    return nc


def _get_nc(bg_scalar=None, mode=None):
    key = None if bg_scalar is None else round(float(bg_scalar), 6)
    if key not in _BUILT:
        _BUILT[key] = _build(key)
    return _BUILT[key]


def _fp8_hi_lo(x):
    import ml_dtypes
    hi = x.astype(ml_dtypes.float8_e4m3)
    lo = (x - hi.astype(np.float32)).astype(ml_dtypes.float8_e4m3)
    return hi, lo


def _prep_host(nodes, owner_masks):
    import ml_dtypes
    fp8 = ml_dtypes.float8_e4m3

    ntot = NCORES * NSH
    nd = np.zeros((ntot, D_IN), fp8)
    nd[:N] = nodes.astype(fp8)
    # ndf[c, g, p, k, n] = nodes[c*NSH + g*GROUP + n, k*128 + p]
    ndr = nd.reshape(NCORES, NGROUPS, GROUP, 2, 128)
    ndT = np.ascontiguousarray(ndr.transpose(0, 1, 4, 3, 2)).reshape(
        NCORES, NGROUPS, 128, 2 * GROUP)

    # masks are 0/1: build fp8 bytes directly (0x00 / 0x38) — exact and fast
    mku = np.zeros((B, ntot), np.uint8)
    np.multiply(owner_masks, np.uint8(0x38), out=mku[:, :N],
                casting="unsafe")
    # mkf[c, g, p, s, b] = mask[b, c*NSH + g*GROUP + s*128 + p]
    mkr = mku.reshape(B, NCORES, NGROUPS, SUBS, 128)
    mkT = np.ascontiguousarray(mkr.transpose(1, 2, 4, 3, 0)).reshape(
        NCORES, NGROUPS, 128, SUBS * B).view(fp8)

    return [(ndT[c], mkT[c]) for c in range(NCORES)]


def kernel(nodes, owner_masks, Wt, bt, Wg, bg, _spmd_extra_kwargs=None):
    import ml_dtypes
    fp8 = ml_dtypes.float8_e4m3

    nodes = np.asarray(nodes, dtype=np.float32)
    owner_masks = np.asarray(owner_masks)
    Wt = np.asarray(Wt, dtype=np.float32)
    bt = np.asarray(bt, dtype=np.float32)
    Wg = np.asarray(Wg, dtype=np.float32)
    bg = np.asarray(bg, dtype=np.float32)

    global _LAST_BG_SCALAR
    _LAST_BG_SCALAR = float(bg[0]) if np.all(bg == bg[0]) else None
    nc = _get_nc(_LAST_BG_SCALAR)

    shards = _prep_host(nodes, owner_masks)

    def wchunks(W):  # [p, k*256 + d] = W[d, k*128 + p]
        w = np.empty((128, 512), np.float32)
        w[:, 0:256] = W.T[0:128]
        w[:, 256:512] = W.T[128:256]
        return w.astype(fp8)

    bt_hi, bt_lo = _fp8_hi_lo(bt)
    bg_hi, bg_lo = _fp8_hi_lo(bg)
    btf = np.empty((1, 2048), fp8)
    btf[0, 0:256] = bt_hi
    btf[0, 256:512] = bt_hi
    btf[0, 512:768] = bg_hi
    btf[0, 768:1024] = bg_hi
    btf[0, 1024:1280] = bt_lo
    btf[0, 1280:1536] = bt_lo
    btf[0, 1536:1792] = bg_lo
    btf[0, 1792:2048] = bg_lo

    wcb = np.concatenate([wchunks(Wt), wchunks(Wg)], axis=1)
    common = {"wcb": wcb, "btf": btf}
    in_maps = [{"ndf": ndTg, "mkf": mkTg, **common}
               for (ndTg, mkTg) in shards]

    extra = _spmd_extra_kwargs or {}
    res = run_bass_kernel_spmd(nc, in_maps, list(range(NCORES)), **extra)
    out = np.zeros((B, D_OUT), np.float64)
    for c in range(NCORES):
        out += res.results[c]["res"].astype(np.float64)
    kernel.last_results = res
    return out.astype(np.float32)
